# revision 5
# baseline (speedup 1.0000x reference)
"""Causal self-attention (B=2, T=2048, C=1024, H=16, D=64) on 8 TRN2 cores.

Sharding: batch across 2 groups of 4 cores; 4 heads per core within a group
(Megatron column-parallel QKV). After attention, AllGather the per-head
outputs within each group, then column-parallel c_proj (each core computes
256 output columns for all T), so the device program is rank-independent.

Per-core dataflow (all matmul operands float32r = full-rate fp32):
  xt  [128, 8, 2048]  x[b]^T chunked by contraction (C) blocks
  Q^T/K^T computed as [512 rows, T] (lhsT = w_qk slices, rhs = xt)
  V computed natural [T, 4*64] with a fused ones column per head
  S^T block matmuls (K=64) row-paired across head pairs (partitions 0-63 /
  64-127), exp on ACT (scale=1/8 fused), triangular mask on diagonal
  128-blocks, att@V with M=65 (row 64 = softmax denominator), reciprocal +
  partition-broadcast normalize, AllGather(y), column-parallel proj.

Output per core: out_c [256, 2048] = out^T columns slice; host reassembles.
"""

import sys

sys.path.insert(0, "/opt/trn_rl_repo")

from contextlib import ExitStack

import numpy as np

B, T, C, H, D = 2, 2048, 1024, 16, 64
NCORES = 8
HL = 4  # heads per core
NKC = 8  # contraction chunks (C / 128)
NCH = 4  # t chunks (T / 512)
NST = 16  # s tiles (T / 128)

_prog_cache = {}


def build_program(reps=1, qk_bias=False, out_bias=False):
    key = (reps, qk_bias, out_bias)
    if key in _prog_cache:
        return _prog_cache[key]

    from concourse import bacc, mybir
    import concourse.tile as tile

    F32 = mybir.dt.float32
    F32R = mybir.dt.float32r
    AF = mybir.ActivationFunctionType
    MUL = mybir.AluOpType.mult

    nc = bacc.Bacc(num_devices=NCORES)

    xt = nc.declare_dram_parameter("xt", [128, NKC, T], F32R, isOutput=False)
    wqk = nc.declare_dram_parameter("wqk", [128, NKC, 512], F32R, isOutput=False)
    wv = nc.declare_dram_parameter("wv", [128, NKC, 256], F32R, isOutput=False)
    wp = nc.declare_dram_parameter("wp", [128, NKC, 256], F32R, isOutput=False)
    tri = nc.declare_dram_parameter("tri", [128, 128], F32R, isOutput=False)
    vone = nc.declare_dram_parameter("vone", [128, NST, 4, 1], F32R, isOutput=False)
    if qk_bias:
        bqk = nc.declare_dram_parameter("bqk", [128, 4], F32, isOutput=False)
    if out_bias:
        bout = nc.declare_dram_parameter("bout", [128, 2], F32, isOutput=False)
    out_c = nc.declare_dram_parameter("out_c", [256, T], F32, isOutput=True)

    with tile.TileContext(nc) as tc:
        with ExitStack() as outer:
            const = outer.enter_context(tc.tile_pool(name="const", bufs=1))
            wqk_sb = const.tile([128, NKC, 512], F32R)
            wv_sb = const.tile([128, NKC, 256], F32R)
            wp_sb = const.tile([128, NKC, 256], F32R)
            tri_sb = const.tile([128, 128], F32R)
            nc.sync.dma_start(wqk_sb[:], wqk[:])
            nc.sync.dma_start(wv_sb[:], wv[:])
            nc.sync.dma_start(wp_sb[:], wp[:])
            nc.sync.dma_start(tri_sb[:], tri[:])
            if qk_bias:
                bqk_sb = const.tile([128, 4], F32)
                nc.sync.dma_start(bqk_sb[:], bqk[:])
            if out_bias:
                bout_sb = const.tile([128, 2], F32)
                nc.sync.dma_start(bout_sb[:], bout[:])

            for rep in range(reps):
                _emit_body(
                    nc, tc, mybir, rep,
                    xt=xt, vone=vone, out_c=out_c,
                    wqk_sb=wqk_sb, wv_sb=wv_sb, wp_sb=wp_sb, tri_sb=tri_sb,
                    bqk_sb=bqk_sb if qk_bias else None,
                    bout_sb=bout_sb if out_bias else None,
                )

    nc.finalize()
    _prog_cache[key] = nc
    return nc


def _emit_body(nc, tc, mybir, rep, *, xt, vone, out_c, wqk_sb, wv_sb, wp_sb,
               tri_sb, bqk_sb, bout_sb):
    F32 = mybir.dt.float32
    F32R = mybir.dt.float32r
    AF = mybir.ActivationFunctionType
    MUL = mybir.AluOpType.mult
    R = f"r{rep}"

    with ExitStack() as persist:
        stP = persist.enter_context(tc.tile_pool(name=f"stP{R}", bufs=1))
        # Q^T/K^T: m-tiles 0,1 = Q pairs; 2,3 = K pairs. [128, m, t]
        qk_sb = stP.tile([128, 4, T], F32R, name=f"qk_sb{R}")
        # V natural, 65-stride per head (65th col = ones)
        v_sb = stP.tile([128, NST, 260], F32R, name=f"v_sb{R}")
        # y raw + denominator row (partition 64), blocks (h*NCH + n)
        yraw = stP.tile([65, 16, 512], F32R, name=f"yraw{R}")

        # ---------------- Stage A: QKV projections ----------------
        with (
            tc.tile_pool(name=f"stA{R}", bufs=1) as stA,
            tc.tile_pool(name=f"psA{R}", bufs=1, space="PSUM") as psA,
        ):
            xt_sb = stA.tile([128, NKC, T], F32R, name=f"xt_sb{R}")
            for n in range(NCH):
                nc.sync.dma_start(
                    xt_sb[:, :, n * 512:(n + 1) * 512],
                    xt[:, :, n * 512:(n + 1) * 512],
                )
            # ones columns of v_sb (positions 65h+64)
            vview = v_sb[:].rearrange("p t (h x) -> p t h x", h=4)
            nc.sync.dma_start(vview[:, :, :, 64:65], vone[:])

            for n in range(NCH):
                for m in range(4):
                    ps = psA.tile([128, 512], F32, tag="qkvps", bufs=2,
                                  name=f"qkvps{R}_{n}_{m}")
                    for kc in range(NKC):
                        nc.tensor.matmul(
                            ps[:],
                            wqk_sb[:, kc, m * 128:(m + 1) * 128],
                            xt_sb[:, kc, n * 512:(n + 1) * 512],
                            start=(kc == 0), stop=(kc == NKC - 1),
                        )
                    if bqk_sb is not None:
                        nc.scalar.activation(
                            qk_sb[:, m, n * 512:(n + 1) * 512], ps[:],
                            AF.Copy, bias=bqk_sb[:, m:m + 1],
                        )
                    else:
                        nc.vector.tensor_copy(
                            qk_sb[:, m, n * 512:(n + 1) * 512], ps[:]
                        )
                for tt in range(4 * n, 4 * n + 4):
                    psv = psA.tile([128, 256], F32, tag="vps", bufs=2,
                                   name=f"vps{R}_{tt}")
                    for kc in range(NKC):
                        nc.tensor.matmul(
                            psv[:],
                            xt_sb[:, kc, tt * 128:(tt + 1) * 128],
                            wv_sb[:, kc, :],
                            start=(kc == 0), stop=(kc == NKC - 1),
                        )
                    nc.vector.tensor_copy(
                        v_sb[:, tt, :].rearrange("p (h x) -> p h x", h=4)[:, :, 0:64],
                        psv[:].rearrange("p (h x) -> p h x", h=4),
                    )

        # ---------------- Stage B: attention ----------------
        with (
            tc.tile_pool(name=f"stB{R}", bufs=1) as stB,
            tc.tile_pool(name=f"psS{R}", bufs=1, space="PSUM") as psS,
            tc.tile_pool(name=f"psY{R}", bufs=1, space="PSUM") as psY,
        ):
            for n in range(NCH):
                for p in range(2):
                    ype = psY.tile([65, 512], F32, tag="ye", bufs=2,
                                   name=f"ype{R}_{n}_{p}")
                    ypo = psY.tile([65, 512], F32, tag="yo", bufs=2,
                                   name=f"ypo{R}_{n}_{p}")
                    n_st = 4 * n + 4
                    for st in range(n_st):
                        diag = st - 4 * n
                        toff = 128 * diag if diag >= 0 else 0
                        pse = psS.tile([128, 512], F32, tag="se", bufs=2,
                                       name=f"pse{R}_{n}_{p}_{st}")
                        pso = psS.tile([128, 512], F32, tag="so", bufs=2,
                                       name=f"pso{R}_{n}_{p}_{st}")
                        ese = stB.tile([128, 512], F32R, tag="ese", bufs=3,
                                       name=f"ese{R}_{n}_{p}_{st}")
                        eso = stB.tile([128, 512], F32R, tag="eso", bufs=3,
                                       name=f"eso{R}_{n}_{p}_{st}")
                        for hp, psu, esu, yp in (
                            (0, pse, ese, ype), (1, pso, eso, ypo)
                        ):
                            pb = 64 * hp
                            h = 2 * p + hp
                            # scores S^T block
                            nc.tensor.matmul(
                                psu[:, toff:512],
                                qk_sb[pb:pb + 64, 2 + p, st * 128:(st + 1) * 128],
                                qk_sb[pb:pb + 64, p, n * 512 + toff:(n + 1) * 512],
                                start=True, stop=True,
                            )
                            nc.scalar.activation(
                                esu[:, toff:512], psu[:, toff:512],
                                AF.Exp, scale=0.125,
                            )
                            if diag >= 0:
                                nc.vector.tensor_tensor(
                                    esu[:, toff:toff + 128],
                                    esu[:, toff:toff + 128],
                                    tri_sb[:], MUL,
                                )
                            nc.tensor.matmul(
                                yp[:, toff:512],
                                v_sb[:, st, 65 * h:65 * h + 65],
                                esu[:, toff:512],
                                start=(st == 0), stop=(st == n_st - 1),
                            )
                    for hp, yp in ((0, ype), (1, ypo)):
                        h = 2 * p + hp
                        nc.vector.tensor_copy(yraw[:, h * NCH + n, :], yp[:])

        # ---------------- Stage C: normalize, AllGather, proj ----------------
        with (
            tc.tile_pool(name=f"stC{R}", bufs=1) as stC,
            tc.tile_pool(name=f"psP{R}", bufs=1, space="PSUM") as psP,
            tc.tile_pool(name=f"dram{R}", bufs=1, space="DRAM") as dpool,
        ):
            den16 = stC.tile([16, 512], F32R, name=f"den16{R}")
            nc.sync.dma_start(den16[:], yraw[64:65, :, :])
            r16 = stC.tile([16, 512], F32, name=f"r16{R}")
            nc.vector.reciprocal(r16[:], den16[:])
            rrow = stC.tile([1, 16, 512], F32, name=f"rrow{R}")
            nc.sync.dma_start(rrow[:], r16[:])

            for h in range(4):
                for n in range(NCH):
                    rb = stC.tile([64, 512], F32, tag="rb", bufs=4,
                                  name=f"rb{R}_{h}_{n}")
                    nc.gpsimd.partition_broadcast(
                        rb[:], rrow[0:1, h * NCH + n, :]
                    )
                    nc.vector.tensor_tensor(
                        yraw[0:64, h * NCH + n, :],
                        yraw[0:64, h * NCH + n, :],
                        rb[:], MUL,
                    )

            y_in = dpool.tile([256, T], F32R, name=f"y_in{R}")
            y_full = dpool.tile([1024, T], F32R, name=f"y_full{R}")
            nc.sync.dma_start(
                y_in[:].rearrange("(h p) (n u) -> p h n u", p=64, n=NCH),
                yraw[0:64, :, :].rearrange("p (h n) u -> p h n u", n=NCH),
            )
            nc.gpsimd.collective_compute(
                "AllGather",
                mybir.AluOpType.bypass,
                replica_groups=[[0, 1, 2, 3], [4, 5, 6, 7]],
                ins=[y_in[:]],
                outs=[y_full[:]],
            )

            pp0 = psP.tile([128, T], F32, name=f"pp0{R}")
            pp1 = psP.tile([128, T], F32, name=f"pp1{R}")
            for kc in range(NKC):
                yf = stC.tile([128, T], F32R, tag="yf", bufs=3,
                              name=f"yf{R}_{kc}")
                nc.sync.dma_start(yf[:], y_full[kc * 128:(kc + 1) * 128, :])
                for m2, pp in ((0, pp0), (1, pp1)):
                    for n4 in range(NCH):
                        nc.tensor.matmul(
                            pp[:, n4 * 512:(n4 + 1) * 512],
                            wp_sb[:, kc, m2 * 128:(m2 + 1) * 128],
                            yf[:, n4 * 512:(n4 + 1) * 512],
                            start=(kc == 0), stop=(kc == NKC - 1),
                        )
            out_sb = stC.tile([128, 2, T], F32, name=f"out_sb{R}")
            for m2, pp in ((0, pp0), (1, pp1)):
                if bout_sb is not None:
                    nc.scalar.activation(
                        out_sb[:, m2, :], pp[:], AF.Copy,
                        bias=bout_sb[:, m2:m2 + 1],
                    )
                else:
                    nc.vector.tensor_copy(out_sb[:, m2, :], pp[:])
            nc.sync.dma_start(
                out_c[:].rearrange("(m p) t -> p m t", p=128), out_sb[:]
            )


def _chunked(a):
    """(C, X) -> [128, C/128, X] contraction-chunked layout."""
    c, x = a.shape
    return np.ascontiguousarray(
        a.reshape(c // 128, 128, x).transpose(1, 0, 2)
    )


def make_in_maps(x, w_attn, b_attn, w_proj, b_proj):
    x = np.asarray(x, dtype=np.float32)
    w_attn = np.asarray(w_attn, dtype=np.float32)
    b_attn = np.asarray(b_attn, dtype=np.float32)
    w_proj = np.asarray(w_proj, dtype=np.float32)
    b_proj = np.asarray(b_proj, dtype=np.float32)

    qk_bias = bool(np.any(b_attn[: 2 * C] != 0))
    b_out_full = b_attn[2 * C:] @ w_proj + b_proj  # V bias folds through
    out_bias = bool(np.any(b_out_full != 0))

    tri_np = np.triu(np.ones((128, 128), np.float32))
    vone_np = np.ones((128, NST, 4, 1), np.float32)
    xt_g = []
    for g in range(B):
        xt_g.append(_chunked(np.ascontiguousarray(x[g].T)))

    in_maps = []
    for core in range(NCORES):
        g, r = core // 4, core % 4
        h0 = r * HL
        qcols = slice(h0 * D, (h0 + HL) * D)
        kcols = slice(C + h0 * D, C + (h0 + HL) * D)
        vcols = slice(2 * C + h0 * D, 2 * C + (h0 + HL) * D)
        wqk_np = _chunked(np.concatenate(
            [w_attn[:, qcols], w_attn[:, kcols]], axis=1))
        wv_np = _chunked(np.ascontiguousarray(w_attn[:, vcols]))
        wp_np = _chunked(np.ascontiguousarray(
            w_proj[:, 256 * r: 256 * (r + 1)]))
        m = {
            "xt": xt_g[g],
            "wqk": wqk_np,
            "wv": wv_np,
            "wp": wp_np,
            "tri": tri_np,
            "vone": vone_np,
        }
        if qk_bias:
            bq = np.concatenate([b_attn[qcols], b_attn[kcols]])  # (512,)
            m["bqk"] = np.ascontiguousarray(
                bq.reshape(4, 128).T.astype(np.float32))
        if out_bias:
            bo = b_out_full[256 * r: 256 * (r + 1)]
            m["bout"] = np.ascontiguousarray(
                bo.reshape(2, 128).T.astype(np.float32))
        in_maps.append(m)
    return in_maps, qk_bias, out_bias


def assemble_output(results):
    out = np.empty((B, T, C), dtype=np.float32)
    for core in range(NCORES):
        g, r = core // 4, core % 4
        out[g][:, 256 * r: 256 * (r + 1)] = results[core]["out_c"].T
    return out


def kernel(x, w_attn, b_attn, w_proj, b_proj):
    from concourse.bass_utils import run_bass_kernel_spmd

    in_maps, qk_bias, out_bias = make_in_maps(
        x, w_attn, b_attn, w_proj, b_proj)
    nc = build_program(reps=1, qk_bias=qk_bias, out_bias=out_bias)
    res = run_bass_kernel_spmd(nc, in_maps, list(range(NCORES)))
    return assemble_output(res.results)


# revision 8
# speedup vs baseline: 1.3253x; 1.3253x over previous
"""Causal self-attention (B=2, T=2048, C=1024, H=16, D=64) on 8 TRN2 cores.

Sharding: batch across 2 groups of 4 cores; 4 heads per core within a group
(Megatron column-parallel QKV). After attention, AllGather the per-head
outputs within each group, then column-parallel c_proj (each core computes
256 output columns for all T), so the device program is rank-independent.

Per-core dataflow (all matmul operands float32r = full-rate fp32):
  xt  [128, 8, 2048]  x[b]^T chunked by contraction (C) blocks
  Q^T/K^T computed as [512 rows, T] (lhsT = w_qk slices, rhs = xt)
  V computed natural [T, 4*64] with a fused ones column per head
  S^T block matmuls (K=64) row-paired across head pairs (partitions 0-63 /
  64-127), exp on ACT (scale=1/8 fused), triangular mask on diagonal
  128-blocks, att@V with M=65 (row 64 = softmax denominator), reciprocal,
  selector-matmul broadcast normalize, AllGather(y), column-parallel proj.

QKV (stage A) and attention (stage B) are emitted interleaved per t-chunk
so the Tile scheduler can fill PE gaps during ACT exp with next-chunk QKV
matmuls. The sequence is split into two t-halves: each half's normalize +
AllGather is issued as soon as its attention chunks finish, so the first
AllGather overlaps the second half's compute; proj runs per half at the
end (PSUM is fully occupied during attention).

Output per core: out_c [256, 2048] = out^T columns slice; host reassembles.
"""

import sys

sys.path.insert(0, "/opt/trn_rl_repo")

from contextlib import ExitStack

import numpy as np

B, T, C, H, D = 2, 2048, 1024, 16, 64
NCORES = 8
HL = 4  # heads per core
NKC = 8  # contraction chunks (C / 128)
NCH = 4  # t chunks (T / 512)
NST = 16  # s tiles (T / 128)
TH = T // 2  # t-half size

_prog_cache = {}


def build_program(reps=1, qk_bias=False, out_bias=False):
    key = (reps, qk_bias, out_bias)
    if key in _prog_cache:
        return _prog_cache[key]

    from concourse import bacc, mybir
    import concourse.tile as tile

    F32 = mybir.dt.float32
    F32R = mybir.dt.float32r

    nc = bacc.Bacc(num_devices=NCORES)

    xt = nc.declare_dram_parameter("xt", [128, NKC, T], F32R, isOutput=False)
    wqk = nc.declare_dram_parameter("wqk", [128, NKC, 512], F32R, isOutput=False)
    wv = nc.declare_dram_parameter("wv", [128, NKC, 256], F32R, isOutput=False)
    wp = nc.declare_dram_parameter("wp", [128, NKC, 256], F32R, isOutput=False)
    tri = nc.declare_dram_parameter("tri", [128, 128], F32R, isOutput=False)
    vone = nc.declare_dram_parameter("vone", [128, NST, 4, 1], F32R, isOutput=False)
    sel = nc.declare_dram_parameter("sel", [16, 16, 64], F32R, isOutput=False)
    if qk_bias:
        bqk = nc.declare_dram_parameter("bqk", [128, 4], F32, isOutput=False)
    if out_bias:
        bout = nc.declare_dram_parameter("bout", [128, 2], F32, isOutput=False)
    out_c = nc.declare_dram_parameter("out_c", [256, T], F32, isOutput=True)

    with tile.TileContext(nc) as tc:
        with ExitStack() as outer:
            const = outer.enter_context(tc.tile_pool(name="const", bufs=1))
            wqk_sb = const.tile([128, NKC, 512], F32R)
            wv_sb = const.tile([128, NKC, 256], F32R)
            wp_sb = const.tile([128, NKC, 256], F32R)
            tri_sb = const.tile([128, 128], F32R)
            sel_sb = const.tile([16, 16, 64], F32R)
            nc.sync.dma_start(wqk_sb[:], wqk[:])
            nc.sync.dma_start(wv_sb[:], wv[:])
            nc.sync.dma_start(wp_sb[:], wp[:])
            nc.sync.dma_start(tri_sb[:], tri[:])
            nc.sync.dma_start(sel_sb[:], sel[:])
            bqk_sb = bout_sb = None
            if qk_bias:
                bqk_sb = const.tile([128, 4], F32)
                nc.sync.dma_start(bqk_sb[:], bqk[:])
            if out_bias:
                bout_sb = const.tile([128, 2], F32)
                nc.sync.dma_start(bout_sb[:], bout[:])

            for rep in range(reps):
                _emit_body(
                    nc, tc, mybir, rep,
                    xt=xt, vone=vone, out_c=out_c,
                    wqk_sb=wqk_sb, wv_sb=wv_sb, wp_sb=wp_sb, tri_sb=tri_sb,
                    sel_sb=sel_sb, bqk_sb=bqk_sb, bout_sb=bout_sb,
                )

    nc.finalize()
    _prog_cache[key] = nc
    return nc


def _emit_body(nc, tc, mybir, rep, *, xt, vone, out_c, wqk_sb, wv_sb, wp_sb,
               tri_sb, sel_sb, bqk_sb, bout_sb):
    F32 = mybir.dt.float32
    F32R = mybir.dt.float32r
    AF = mybir.ActivationFunctionType
    MUL = mybir.AluOpType.mult
    R = f"r{rep}"

    with ExitStack() as persist:
        stP = persist.enter_context(tc.tile_pool(name=f"stP{R}", bufs=1))
        dpool = persist.enter_context(
            tc.tile_pool(name=f"dram{R}", bufs=1, space="DRAM"))
        # Q^T/K^T: m-tiles 0,1 = Q pairs; 2,3 = K pairs. [128, m, t]
        qk_sb = stP.tile([128, 4, T], F32R, name=f"qk_sb{R}")
        # V natural, 65-stride per head (65th col = ones)
        v_sb = stP.tile([128, NST, 260], F32R, name=f"v_sb{R}")
        # y raw + denominator row (partition 64), per half; blocks (h*2 + nl)
        yraw_h = [
            stP.tile([65, 8, 512], F32R, name=f"yraw{R}_{hf}")
            for hf in range(2)
        ]
        y_in_h = [
            dpool.tile([256, TH], F32R, name=f"y_in{R}_{hf}")
            for hf in range(2)
        ]
        y_full_h = [
            dpool.tile([1024, TH], F32R, name=f"y_full{R}_{hf}")
            for hf in range(2)
        ]

        with (
            tc.tile_pool(name=f"stAB{R}", bufs=1) as stAB,
            tc.tile_pool(name=f"psA{R}", bufs=1, space="PSUM") as psA,
            tc.tile_pool(name=f"psS{R}", bufs=1, space="PSUM") as psS,
            tc.tile_pool(name=f"psY{R}", bufs=1, space="PSUM") as psY,
        ):
            xt_sb = stAB.tile([128, NKC, T], F32R, name=f"xt_sb{R}")
            vview = v_sb[:].rearrange("p t (h x) -> p t h x", h=4)
            nc.sync.dma_start(vview[:, :, :, 64:65], vone[:])

            for half in range(2):
                yraw = yraw_h[half]
                for n in (2 * half, 2 * half + 1):
                    nl = n - 2 * half
                    # ---- stage A for chunk n ----
                    nc.sync.dma_start(
                        xt_sb[:, :, n * 512:(n + 1) * 512],
                        xt[:, :, n * 512:(n + 1) * 512],
                    )
                    for m in range(4):
                        ps = psA.tile([128, 512], F32, tag="pA", bufs=2,
                                      name=f"qkvps{R}_{n}_{m}")
                        for kc in range(NKC):
                            nc.tensor.matmul(
                                ps[:],
                                wqk_sb[:, kc, m * 128:(m + 1) * 128],
                                xt_sb[:, kc, n * 512:(n + 1) * 512],
                                start=(kc == 0), stop=(kc == NKC - 1),
                            )
                        if bqk_sb is not None:
                            nc.scalar.activation(
                                qk_sb[:, m, n * 512:(n + 1) * 512], ps[:],
                                AF.Copy, bias=bqk_sb[:, m:m + 1],
                            )
                        else:
                            nc.vector.tensor_copy(
                                qk_sb[:, m, n * 512:(n + 1) * 512], ps[:]
                            )
                    for tt in range(4 * n, 4 * n + 4):
                        psv = psA.tile([128, 512], F32, tag="pA", bufs=2,
                                       name=f"vps{R}_{tt}")
                        for kc in range(NKC):
                            nc.tensor.matmul(
                                psv[:, 0:256],
                                xt_sb[:, kc, tt * 128:(tt + 1) * 128],
                                wv_sb[:, kc, :],
                                start=(kc == 0), stop=(kc == NKC - 1),
                            )
                        nc.vector.tensor_copy(
                            v_sb[:, tt, :].rearrange(
                                "p (h x) -> p h x", h=4)[:, :, 0:64],
                            psv[:, 0:256].rearrange("p (h x) -> p h x", h=4),
                        )

                    # ---- stage B for chunk n ----
                    n_st = 4 * n + 4
                    for p in range(2):
                        ype = psY.tile([65, 512], F32, tag="ye", bufs=1,
                                       name=f"ype{R}_{n}_{p}")
                        ypo = psY.tile([65, 512], F32, tag="yo", bufs=1,
                                       name=f"ypo{R}_{n}_{p}")
                        for st in range(n_st):
                            diag = st - 4 * n
                            toff = 128 * diag if diag >= 0 else 0
                            scp = psS.tile([128, 1024], F32, tag="sc", bufs=2,
                                           name=f"scp{R}_{n}_{p}_{st}")
                            es = stAB.tile([128, 1024], F32R, tag="es", bufs=3,
                                           name=f"es{R}_{n}_{p}_{st}")
                            for hp in range(2):
                                pb = 64 * hp
                                nc.tensor.matmul(
                                    scp[:, hp * 512 + toff:(hp + 1) * 512],
                                    qk_sb[pb:pb + 64, 2 + p,
                                          st * 128:(st + 1) * 128],
                                    qk_sb[pb:pb + 64, p,
                                          n * 512 + toff:(n + 1) * 512],
                                    start=True, stop=True,
                                )
                            if diag < 0:
                                nc.scalar.activation(
                                    es[:], scp[:], AF.Exp, scale=0.125
                                )
                            else:
                                for hp in range(2):
                                    nc.scalar.activation(
                                        es[:, hp * 512 + toff:(hp + 1) * 512],
                                        scp[:, hp * 512 + toff:(hp + 1) * 512],
                                        AF.Exp, scale=0.125,
                                    )
                                for hp in range(2):
                                    nc.vector.tensor_tensor(
                                        es[:, hp * 512 + toff:
                                           hp * 512 + toff + 128],
                                        es[:, hp * 512 + toff:
                                           hp * 512 + toff + 128],
                                        tri_sb[:], MUL,
                                    )
                            for hp, yp in ((0, ype), (1, ypo)):
                                h = 2 * p + hp
                                nc.tensor.matmul(
                                    yp[:, toff:512],
                                    v_sb[:, st, 65 * h:65 * h + 65],
                                    es[:, hp * 512 + toff:(hp + 1) * 512],
                                    start=(st == 0), stop=(st == n_st - 1),
                                )
                        for hp, yp in ((0, ype), (1, ypo)):
                            h = 2 * p + hp
                            nc.vector.tensor_copy(
                                yraw[:, h * 2 + nl, :], yp[:]
                            )

                # ---- normalize + AllGather for this half ----
                den8 = stAB.tile([8, 512], F32R, tag="den8", bufs=2,
                                 name=f"den8{R}_{half}")
                nc.sync.dma_start(den8[:], yraw[64:65, :, :])
                r8 = stAB.tile([8, 512], F32R, tag="r8", bufs=2,
                               name=f"r8{R}_{half}")
                with nc.allow_low_precision(reason="f32r softmax denom recip"):
                    nc.vector.reciprocal(r8[:], den8[:])
                for h in range(4):
                    for nl in range(2):
                        idx = h * 2 + nl
                        rb = psA.tile([64, 512], F32, tag="pA", bufs=2,
                                      name=f"rb{R}_{half}_{h}_{nl}")
                        nc.tensor.matmul(
                            rb[:], sel_sb[0:8, idx, :], r8[:],
                            start=True, stop=True,
                        )
                        nc.vector.tensor_tensor(
                            yraw[0:64, idx, :], yraw[0:64, idx, :],
                            rb[:], MUL,
                        )
                nc.sync.dma_start(
                    y_in_h[half][:].rearrange(
                        "(h p) (nl u) -> p h nl u", p=64, nl=2),
                    yraw[0:64, :, :].rearrange(
                        "p (h nl) u -> p h nl u", nl=2),
                )
                nc.gpsimd.collective_compute(
                    "AllGather",
                    mybir.AluOpType.bypass,
                    replica_groups=[[0, 1, 2, 3], [4, 5, 6, 7]],
                    ins=[y_in_h[half][:]],
                    outs=[y_full_h[half][:]],
                )

        # ------- proj per half -------
        with (
            tc.tile_pool(name=f"stC{R}", bufs=1) as stC,
            tc.tile_pool(name=f"psP{R}", bufs=1, space="PSUM") as psP,
        ):
            for half in range(2):
                pp0 = psP.tile([128, TH], F32, tag="pp0", bufs=1,
                               name=f"pp0{R}_{half}")
                pp1 = psP.tile([128, TH], F32, tag="pp1", bufs=1,
                               name=f"pp1{R}_{half}")
                for kc in range(NKC):
                    yf = stC.tile([128, TH], F32R, tag="yf", bufs=4,
                                  name=f"yf{R}_{half}_{kc}")
                    nc.sync.dma_start(
                        yf[:], y_full_h[half][kc * 128:(kc + 1) * 128, :]
                    )
                    for m2, pp in ((0, pp0), (1, pp1)):
                        for n4 in range(2):
                            nc.tensor.matmul(
                                pp[:, n4 * 512:(n4 + 1) * 512],
                                wp_sb[:, kc, m2 * 128:(m2 + 1) * 128],
                                yf[:, n4 * 512:(n4 + 1) * 512],
                                start=(kc == 0), stop=(kc == NKC - 1),
                            )
                out_sb = stC.tile([128, 2, TH], F32, tag="out_sb", bufs=2,
                                  name=f"out_sb{R}_{half}")
                for m2, pp in ((0, pp0), (1, pp1)):
                    if bout_sb is not None:
                        nc.scalar.activation(
                            out_sb[:, m2, :], pp[:], AF.Copy,
                            bias=bout_sb[:, m2:m2 + 1],
                        )
                    else:
                        nc.vector.tensor_copy(out_sb[:, m2, :], pp[:])
                nc.sync.dma_start(
                    out_c[:, half * TH:(half + 1) * TH].rearrange(
                        "(m p) t -> p m t", p=128),
                    out_sb[:],
                )


def _chunked(a):
    """(C, X) -> [128, C/128, X] contraction-chunked layout."""
    c, x = a.shape
    return np.ascontiguousarray(
        a.reshape(c // 128, 128, x).transpose(1, 0, 2)
    )


def make_in_maps(x, w_attn, b_attn, w_proj, b_proj):
    x = np.asarray(x, dtype=np.float32)
    w_attn = np.asarray(w_attn, dtype=np.float32)
    b_attn = np.asarray(b_attn, dtype=np.float32)
    w_proj = np.asarray(w_proj, dtype=np.float32)
    b_proj = np.asarray(b_proj, dtype=np.float32)

    qk_bias = bool(np.any(b_attn[: 2 * C] != 0))
    b_out_full = b_attn[2 * C:] @ w_proj + b_proj  # V bias folds through
    out_bias = bool(np.any(b_out_full != 0))

    tri_np = np.triu(np.ones((128, 128), np.float32))
    vone_np = np.ones((128, NST, 4, 1), np.float32)
    sel_np = np.zeros((16, 16, 64), np.float32)
    for i in range(16):
        sel_np[i, i, :] = 1.0
    xt_g = []
    for g in range(B):
        xt_g.append(_chunked(np.ascontiguousarray(x[g].T)))

    in_maps = []
    for core in range(NCORES):
        g, r = core // 4, core % 4
        h0 = r * HL
        qcols = slice(h0 * D, (h0 + HL) * D)
        kcols = slice(C + h0 * D, C + (h0 + HL) * D)
        vcols = slice(2 * C + h0 * D, 2 * C + (h0 + HL) * D)
        wqk_np = _chunked(np.concatenate(
            [w_attn[:, qcols], w_attn[:, kcols]], axis=1))
        wv_np = _chunked(np.ascontiguousarray(w_attn[:, vcols]))
        wp_np = _chunked(np.ascontiguousarray(
            w_proj[:, 256 * r: 256 * (r + 1)]))
        m = {
            "xt": xt_g[g],
            "wqk": wqk_np,
            "wv": wv_np,
            "wp": wp_np,
            "tri": tri_np,
            "vone": vone_np,
            "sel": sel_np,
        }
        if qk_bias:
            bq = np.concatenate([b_attn[qcols], b_attn[kcols]])  # (512,)
            m["bqk"] = np.ascontiguousarray(
                bq.reshape(4, 128).T.astype(np.float32))
        if out_bias:
            bo = b_out_full[256 * r: 256 * (r + 1)]
            m["bout"] = np.ascontiguousarray(
                bo.reshape(2, 128).T.astype(np.float32))
        in_maps.append(m)
    return in_maps, qk_bias, out_bias


def assemble_output(results):
    out = np.empty((B, T, C), dtype=np.float32)
    for core in range(NCORES):
        g, r = core // 4, core % 4
        out[g][:, 256 * r: 256 * (r + 1)] = results[core]["out_c"].T
    return out


def kernel(x, w_attn, b_attn, w_proj, b_proj):
    from concourse.bass_utils import run_bass_kernel_spmd

    in_maps, qk_bias, out_bias = make_in_maps(
        x, w_attn, b_attn, w_proj, b_proj)
    nc = build_program(reps=1, qk_bias=qk_bias, out_bias=out_bias)
    res = run_bass_kernel_spmd(nc, in_maps, list(range(NCORES)))
    return assemble_output(res.results)


# revision 9
# speedup vs baseline: 1.7042x; 1.2859x over previous
"""Causal self-attention (B=2, T=2048, C=1024, H=16, D=64) on 8 TRN2 cores.

Sharding: batch across 2 groups of 4 cores; 4 heads per core within a group
(Megatron column-parallel QKV). After attention, AllGather the per-head
outputs within each group, then column-parallel c_proj (each core computes
256 output columns for all T), so the device program is rank-independent.

Per-core dataflow (all matmul operands float32r = full-rate fp32):
  xt  [128, 8, 2048]  x[b]^T chunked by contraction (C) blocks
  Q^T/K^T computed as [512 rows, T] (lhsT = w_qk slices, rhs = xt)
  V computed natural [T, 4*64] with a fused ones column per head
  S^T block matmuls (K=64) row-paired across head pairs (partitions 0-63 /
  64-127), exp on ACT (scale=1/8 fused), triangular mask on diagonal
  128-blocks, att@V with M=65 (row 64 = softmax denominator), reciprocal,
  selector-matmul broadcast normalize, AllGather(y), column-parallel proj.

QKV (stage A) and attention (stage B) are emitted interleaved per t-chunk
so the Tile scheduler can fill PE gaps during ACT exp with next-chunk QKV
matmuls. The sequence is split into two t-halves: each half's normalize +
AllGather is issued as soon as its attention chunks finish, so the first
AllGather overlaps the second half's compute; proj runs per half at the
end (PSUM is fully occupied during attention).

Output per core: out_c [256, 2048] = out^T columns slice; host reassembles.
"""

import sys

sys.path.insert(0, "/opt/trn_rl_repo")

from contextlib import ExitStack

import numpy as np

B, T, C, H, D = 2, 2048, 1024, 16, 64
NCORES = 8
HL = 4  # heads per core
NKC = 8  # contraction chunks (C / 128)
NCH = 4  # t chunks (T / 512)
NST = 16  # s tiles (T / 128)
TH = T // 2  # t-half size

_prog_cache = {}


def build_program(reps=1, qk_bias=False, out_bias=False):
    key = (reps, qk_bias, out_bias)
    if key in _prog_cache:
        return _prog_cache[key]

    from concourse import bacc, mybir
    import concourse.tile as tile

    F32 = mybir.dt.float32
    F32R = mybir.dt.float32r

    nc = bacc.Bacc(num_devices=NCORES)

    xt = nc.declare_dram_parameter("xt", [128, NKC, T], F32R, isOutput=False)
    wqk = nc.declare_dram_parameter("wqk", [128, NKC, 512], F32R, isOutput=False)
    wv = nc.declare_dram_parameter("wv", [128, NKC, 256], F32R, isOutput=False)
    wp = nc.declare_dram_parameter("wp", [128, NKC, 256], F32R, isOutput=False)
    tri = nc.declare_dram_parameter("tri", [128, 128], F32R, isOutput=False)
    vone = nc.declare_dram_parameter("vone", [128, NST, 4, 1], F32R, isOutput=False)
    sel = nc.declare_dram_parameter("sel", [16, 16, 64], F32R, isOutput=False)
    if qk_bias:
        bqk = nc.declare_dram_parameter("bqk", [128, 4], F32, isOutput=False)
    if out_bias:
        bout = nc.declare_dram_parameter("bout", [128, 2], F32, isOutput=False)
    out_c = nc.declare_dram_parameter("out_c", [256, T], F32, isOutput=True)

    with tile.TileContext(nc) as tc:
        with ExitStack() as outer:
            const = outer.enter_context(tc.tile_pool(name="const", bufs=1))
            wqk_sb = const.tile([128, NKC, 512], F32R)
            wv_sb = const.tile([128, NKC, 256], F32R)
            wp_sb = const.tile([128, NKC, 256], F32R)
            tri_sb = const.tile([128, 128], F32R)
            sel_sb = const.tile([16, 16, 64], F32R)
            nc.scalar.dma_start(wqk_sb[:], wqk[:])
            nc.scalar.dma_start(wv_sb[:], wv[:])
            nc.scalar.dma_start(wp_sb[:], wp[:])
            nc.scalar.dma_start(tri_sb[:], tri[:])
            nc.scalar.dma_start(sel_sb[:], sel[:])
            bqk_sb = bout_sb = None
            if qk_bias:
                bqk_sb = const.tile([128, 4], F32)
                nc.scalar.dma_start(bqk_sb[:], bqk[:])
            if out_bias:
                bout_sb = const.tile([128, 2], F32)
                nc.scalar.dma_start(bout_sb[:], bout[:])

            for rep in range(reps):
                _emit_body(
                    nc, tc, mybir, rep,
                    xt=xt, vone=vone, out_c=out_c,
                    wqk_sb=wqk_sb, wv_sb=wv_sb, wp_sb=wp_sb, tri_sb=tri_sb,
                    sel_sb=sel_sb, bqk_sb=bqk_sb, bout_sb=bout_sb,
                )

    nc.finalize()
    _prog_cache[key] = nc
    return nc


def _emit_body(nc, tc, mybir, rep, *, xt, vone, out_c, wqk_sb, wv_sb, wp_sb,
               tri_sb, sel_sb, bqk_sb, bout_sb):
    F32 = mybir.dt.float32
    F32R = mybir.dt.float32r
    AF = mybir.ActivationFunctionType
    MUL = mybir.AluOpType.mult
    R = f"r{rep}"

    with ExitStack() as persist:
        stP = persist.enter_context(tc.tile_pool(name=f"stP{R}", bufs=1))
        dpool = persist.enter_context(
            tc.tile_pool(name=f"dram{R}", bufs=1, space="DRAM"))
        # Q^T/K^T: m-tiles 0,1 = Q pairs; 2,3 = K pairs. [128, m, t]
        qk_sb = stP.tile([128, 4, T], F32R, name=f"qk_sb{R}")
        # V natural, 65-stride per head (65th col = ones)
        v_sb = stP.tile([128, NST, 260], F32R, name=f"v_sb{R}")
        # y raw + denominator row (partition 64), per half; blocks (h*2 + nl)
        yraw_h = [
            stP.tile([65, 8, 512], F32R, name=f"yraw{R}_{hf}")
            for hf in range(2)
        ]
        y_in_h = [
            dpool.tile([256, TH], F32R, name=f"y_in{R}_{hf}")
            for hf in range(2)
        ]
        y_full_h = [
            dpool.tile([1024, TH], F32R, name=f"y_full{R}_{hf}")
            for hf in range(2)
        ]

        with (
            tc.tile_pool(name=f"stAB{R}", bufs=1) as stAB,
            tc.tile_pool(name=f"psA{R}", bufs=1, space="PSUM") as psA,
            tc.tile_pool(name=f"psS{R}", bufs=1, space="PSUM") as psS,
            tc.tile_pool(name=f"psY{R}", bufs=1, space="PSUM") as psY,
        ):
            xt_sb = stAB.tile([128, NKC, T], F32R, name=f"xt_sb{R}")
            vview = v_sb[:].rearrange("p t (h x) -> p t h x", h=4)
            nc.scalar.dma_start(vview[:, :, :, 64:65], vone[:])
            for n in range(NCH):
                nc.sync.dma_start(
                    xt_sb[:, :, n * 512:(n + 1) * 512],
                    xt[:, :, n * 512:(n + 1) * 512],
                )

            for half in range(2):
                yraw = yraw_h[half]
                for n in (2 * half, 2 * half + 1):
                    nl = n - 2 * half
                    for m in range(4):
                        ps = psA.tile([128, 512], F32, tag="pA", bufs=2,
                                      name=f"qkvps{R}_{n}_{m}")
                        for kc in range(NKC):
                            nc.tensor.matmul(
                                ps[:],
                                wqk_sb[:, kc, m * 128:(m + 1) * 128],
                                xt_sb[:, kc, n * 512:(n + 1) * 512],
                                start=(kc == 0), stop=(kc == NKC - 1),
                            )
                        if bqk_sb is not None:
                            nc.scalar.activation(
                                qk_sb[:, m, n * 512:(n + 1) * 512], ps[:],
                                AF.Copy, bias=bqk_sb[:, m:m + 1],
                            )
                        else:
                            nc.vector.tensor_copy(
                                qk_sb[:, m, n * 512:(n + 1) * 512], ps[:]
                            )
                    for tt in range(4 * n, 4 * n + 4):
                        psv = psA.tile([128, 512], F32, tag="pA", bufs=2,
                                       name=f"vps{R}_{tt}")
                        for kc in range(NKC):
                            nc.tensor.matmul(
                                psv[:, 0:256],
                                xt_sb[:, kc, tt * 128:(tt + 1) * 128],
                                wv_sb[:, kc, :],
                                start=(kc == 0), stop=(kc == NKC - 1),
                            )
                        nc.vector.tensor_copy(
                            v_sb[:, tt, :].rearrange(
                                "p (h x) -> p h x", h=4)[:, :, 0:64],
                            psv[:, 0:256].rearrange("p (h x) -> p h x", h=4),
                        )

                    # ---- stage B for chunk n ----
                    n_st = 4 * n + 4
                    for p in range(2):
                        ype = psY.tile([65, 512], F32, tag="ye", bufs=1,
                                       name=f"ype{R}_{n}_{p}")
                        ypo = psY.tile([65, 512], F32, tag="yo", bufs=1,
                                       name=f"ypo{R}_{n}_{p}")
                        for st in range(n_st):
                            diag = st - 4 * n
                            toff = 128 * diag if diag >= 0 else 0
                            scp = psS.tile([128, 1024], F32, tag="sc", bufs=2,
                                           name=f"scp{R}_{n}_{p}_{st}")
                            es = stAB.tile([128, 1024], F32R, tag="es", bufs=3,
                                           name=f"es{R}_{n}_{p}_{st}")
                            for hp in range(2):
                                pb = 64 * hp
                                nc.tensor.matmul(
                                    scp[:, hp * 512 + toff:(hp + 1) * 512],
                                    qk_sb[pb:pb + 64, 2 + p,
                                          st * 128:(st + 1) * 128],
                                    qk_sb[pb:pb + 64, p,
                                          n * 512 + toff:(n + 1) * 512],
                                    start=True, stop=True,
                                )
                            if diag < 0:
                                nc.scalar.activation(
                                    es[:], scp[:], AF.Exp, scale=0.125
                                )
                            else:
                                for hp in range(2):
                                    nc.scalar.activation(
                                        es[:, hp * 512 + toff:(hp + 1) * 512],
                                        scp[:, hp * 512 + toff:(hp + 1) * 512],
                                        AF.Exp, scale=0.125,
                                    )
                                for hp in range(2):
                                    nc.vector.tensor_tensor(
                                        es[:, hp * 512 + toff:
                                           hp * 512 + toff + 128],
                                        es[:, hp * 512 + toff:
                                           hp * 512 + toff + 128],
                                        tri_sb[:], MUL,
                                    )
                            for hp, yp in ((0, ype), (1, ypo)):
                                h = 2 * p + hp
                                nc.tensor.matmul(
                                    yp[:, toff:512],
                                    v_sb[:, st, 65 * h:65 * h + 65],
                                    es[:, hp * 512 + toff:(hp + 1) * 512],
                                    start=(st == 0), stop=(st == n_st - 1),
                                )
                        for hp, yp in ((0, ype), (1, ypo)):
                            h = 2 * p + hp
                            nc.vector.tensor_copy(
                                yraw[:, h * 2 + nl, :], yp[:]
                            )

                # ---- normalize + AllGather for this half ----
                den8 = stAB.tile([8, 512], F32R, tag="den8", bufs=2,
                                 name=f"den8{R}_{half}")
                nc.scalar.dma_start(den8[:], yraw[64:65, :, :])
                r8 = stAB.tile([8, 512], F32R, tag="r8", bufs=2,
                               name=f"r8{R}_{half}")
                with nc.allow_low_precision(reason="f32r softmax denom recip"):
                    nc.vector.reciprocal(r8[:], den8[:])
                for h in range(4):
                    for nl in range(2):
                        idx = h * 2 + nl
                        rb = psA.tile([64, 512], F32, tag="pA", bufs=2,
                                      name=f"rb{R}_{half}_{h}_{nl}")
                        nc.tensor.matmul(
                            rb[:], sel_sb[0:8, idx, :], r8[:],
                            start=True, stop=True,
                        )
                        nc.vector.tensor_tensor(
                            yraw[0:64, idx, :], yraw[0:64, idx, :],
                            rb[:], MUL,
                        )
                nc.scalar.dma_start(
                    y_in_h[half][:].rearrange(
                        "(h p) (nl u) -> p h nl u", p=64, nl=2),
                    yraw[0:64, :, :].rearrange(
                        "p (h nl) u -> p h nl u", nl=2),
                )
                nc.gpsimd.collective_compute(
                    "AllGather",
                    mybir.AluOpType.bypass,
                    replica_groups=[[0, 1, 2, 3], [4, 5, 6, 7]],
                    ins=[y_in_h[half][:]],
                    outs=[y_full_h[half][:]],
                )

        # ------- proj per half -------
        with (
            tc.tile_pool(name=f"stC{R}", bufs=1) as stC,
            tc.tile_pool(name=f"psP{R}", bufs=1, space="PSUM") as psP,
        ):
            for half in range(2):
                pp0 = psP.tile([128, TH], F32, tag="pp0", bufs=1,
                               name=f"pp0{R}_{half}")
                pp1 = psP.tile([128, TH], F32, tag="pp1", bufs=1,
                               name=f"pp1{R}_{half}")
                for kc in range(NKC):
                    yf = stC.tile([128, TH], F32R, tag="yf", bufs=4,
                                  name=f"yf{R}_{half}_{kc}")
                    dma_eng = nc.sync if kc % 2 == 0 else nc.scalar
                    dma_eng.dma_start(
                        yf[:], y_full_h[half][kc * 128:(kc + 1) * 128, :]
                    )
                    for m2, pp in ((0, pp0), (1, pp1)):
                        for n4 in range(2):
                            nc.tensor.matmul(
                                pp[:, n4 * 512:(n4 + 1) * 512],
                                wp_sb[:, kc, m2 * 128:(m2 + 1) * 128],
                                yf[:, n4 * 512:(n4 + 1) * 512],
                                start=(kc == 0), stop=(kc == NKC - 1),
                            )
                out_sb = stC.tile([128, 2, TH], F32, tag="out_sb", bufs=2,
                                  name=f"out_sb{R}_{half}")
                for m2, pp in ((0, pp0), (1, pp1)):
                    if bout_sb is not None:
                        nc.scalar.activation(
                            out_sb[:, m2, :], pp[:], AF.Copy,
                            bias=bout_sb[:, m2:m2 + 1],
                        )
                    else:
                        nc.vector.tensor_copy(out_sb[:, m2, :], pp[:])
                nc.sync.dma_start(
                    out_c[:, half * TH:(half + 1) * TH].rearrange(
                        "(m p) t -> p m t", p=128),
                    out_sb[:],
                )


def _chunked(a):
    """(C, X) -> [128, C/128, X] contraction-chunked layout."""
    c, x = a.shape
    return np.ascontiguousarray(
        a.reshape(c // 128, 128, x).transpose(1, 0, 2)
    )


def make_in_maps(x, w_attn, b_attn, w_proj, b_proj):
    x = np.asarray(x, dtype=np.float32)
    w_attn = np.asarray(w_attn, dtype=np.float32)
    b_attn = np.asarray(b_attn, dtype=np.float32)
    w_proj = np.asarray(w_proj, dtype=np.float32)
    b_proj = np.asarray(b_proj, dtype=np.float32)

    qk_bias = bool(np.any(b_attn[: 2 * C] != 0))
    b_out_full = b_attn[2 * C:] @ w_proj + b_proj  # V bias folds through
    out_bias = bool(np.any(b_out_full != 0))

    tri_np = np.triu(np.ones((128, 128), np.float32))
    vone_np = np.ones((128, NST, 4, 1), np.float32)
    sel_np = np.zeros((16, 16, 64), np.float32)
    for i in range(16):
        sel_np[i, i, :] = 1.0
    xt_g = []
    for g in range(B):
        xt_g.append(_chunked(np.ascontiguousarray(x[g].T)))

    in_maps = []
    for core in range(NCORES):
        g, r = core // 4, core % 4
        h0 = r * HL
        qcols = slice(h0 * D, (h0 + HL) * D)
        kcols = slice(C + h0 * D, C + (h0 + HL) * D)
        vcols = slice(2 * C + h0 * D, 2 * C + (h0 + HL) * D)
        wqk_np = _chunked(np.concatenate(
            [w_attn[:, qcols], w_attn[:, kcols]], axis=1))
        wv_np = _chunked(np.ascontiguousarray(w_attn[:, vcols]))
        wp_np = _chunked(np.ascontiguousarray(
            w_proj[:, 256 * r: 256 * (r + 1)]))
        m = {
            "xt": xt_g[g],
            "wqk": wqk_np,
            "wv": wv_np,
            "wp": wp_np,
            "tri": tri_np,
            "vone": vone_np,
            "sel": sel_np,
        }
        if qk_bias:
            bq = np.concatenate([b_attn[qcols], b_attn[kcols]])  # (512,)
            m["bqk"] = np.ascontiguousarray(
                bq.reshape(4, 128).T.astype(np.float32))
        if out_bias:
            bo = b_out_full[256 * r: 256 * (r + 1)]
            m["bout"] = np.ascontiguousarray(
                bo.reshape(2, 128).T.astype(np.float32))
        in_maps.append(m)
    return in_maps, qk_bias, out_bias


def assemble_output(results):
    out = np.empty((B, T, C), dtype=np.float32)
    for core in range(NCORES):
        g, r = core // 4, core % 4
        out[g][:, 256 * r: 256 * (r + 1)] = results[core]["out_c"].T
    return out


def kernel(x, w_attn, b_attn, w_proj, b_proj):
    from concourse.bass_utils import run_bass_kernel_spmd

    in_maps, qk_bias, out_bias = make_in_maps(
        x, w_attn, b_attn, w_proj, b_proj)
    nc = build_program(reps=1, qk_bias=qk_bias, out_bias=out_bias)
    res = run_bass_kernel_spmd(nc, in_maps, list(range(NCORES)))
    return assemble_output(res.results)


# revision 12
# speedup vs baseline: 2.1217x; 1.2450x over previous
"""Causal self-attention (B=2, T=2048, C=1024, H=16, D=64) on 8 TRN2 cores.

Sharding: batch across 2 groups of 4 cores; 4 heads per core within a group
(Megatron column-parallel QKV). After attention, AllGather the per-head
outputs within each group, then column-parallel c_proj (each core computes
256 output columns for all T), so the device program is rank-independent.

Per-core dataflow (all matmul operands float32r = full-rate fp32):
  xt  [128, 8, 2048]  x[b]^T chunked by contraction (C) blocks
  Q^T/K^T computed as [512 rows, T] (lhsT = w_qk slices, rhs = xt)
  V computed natural [T, 4*64] with a fused ones column per head
  S^T block matmuls (K=64) row-paired across head pairs (partitions 0-63 /
  64-127), exp on ACT (scale=1/8 fused), triangular mask on diagonal
  128-blocks, att@V with M=65 (row 64 = softmax denominator), reciprocal,
  selector-matmul broadcast normalize, AllGather(y), column-parallel proj.

QKV (stage A) and attention (stage B) are emitted interleaved per t-chunk
so the Tile scheduler can fill PE gaps during ACT exp with next-chunk QKV
matmuls. The sequence is split into two t-halves: each half's normalize +
AllGather is issued as soon as its attention chunks finish, so the first
AllGather overlaps the second half's compute; proj runs per half at the
end (PSUM is fully occupied during attention).

Output per core: out_c [256, 2048] = out^T columns slice; host reassembles.
"""

import sys

sys.path.insert(0, "/opt/trn_rl_repo")

from contextlib import ExitStack

import numpy as np

B, T, C, H, D = 2, 2048, 1024, 16, 64
NCORES = 8
HL = 4  # heads per core
NKC = 8  # contraction chunks (C / 128)
NCH = 4  # t chunks (T / 512)
NST = 16  # s tiles (T / 128)
TH = T // 2  # t-half size

_prog_cache = {}


def build_program(reps=1, qk_bias=False, out_bias=False):
    key = (reps, qk_bias, out_bias)
    if key in _prog_cache:
        return _prog_cache[key]

    from concourse import bacc, mybir
    import concourse.tile as tile

    F32 = mybir.dt.float32
    F32R = mybir.dt.float32r

    nc = bacc.Bacc(num_devices=NCORES)

    xt = nc.declare_dram_parameter("xt", [128, NKC, T], F32R, isOutput=False)
    wqk = nc.declare_dram_parameter("wqk", [128, NKC, 512], F32R, isOutput=False)
    wv = nc.declare_dram_parameter("wv", [128, NKC, 256], F32R, isOutput=False)
    wp = nc.declare_dram_parameter("wp", [128, NKC, 256], F32R, isOutput=False)
    tri = nc.declare_dram_parameter("tri", [128, 128], F32R, isOutput=False)
    vone = nc.declare_dram_parameter("vone", [128, NST, 4, 1], F32R, isOutput=False)
    sel = nc.declare_dram_parameter("sel", [16, 16, 64], F32R, isOutput=False)
    if qk_bias:
        bqk = nc.declare_dram_parameter("bqk", [128, 4], F32, isOutput=False)
    if out_bias:
        bout = nc.declare_dram_parameter("bout", [128, 2], F32, isOutput=False)
    out_c = nc.declare_dram_parameter("out_c", [256, T], F32, isOutput=True)

    with tile.TileContext(nc) as tc:
        with ExitStack() as outer:
            const = outer.enter_context(tc.tile_pool(name="const", bufs=1))
            wqk_sb = const.tile([128, NKC, 512], F32R)
            wv_sb = const.tile([128, NKC, 256], F32R)
            wp_sb = const.tile([128, NKC, 256], F32R)
            tri_sb = const.tile([128, 128], F32R)
            sel_sb = const.tile([16, 16, 64], F32R)
            nc.scalar.dma_start(wqk_sb[:], wqk[:])
            nc.scalar.dma_start(wv_sb[:], wv[:])
            nc.scalar.dma_start(wp_sb[:], wp[:])
            nc.scalar.dma_start(tri_sb[:], tri[:])
            nc.scalar.dma_start(sel_sb[:], sel[:])
            bqk_sb = bout_sb = None
            if qk_bias:
                bqk_sb = const.tile([128, 4], F32)
                nc.scalar.dma_start(bqk_sb[:], bqk[:])
            if out_bias:
                bout_sb = const.tile([128, 2], F32)
                nc.scalar.dma_start(bout_sb[:], bout[:])

            for rep in range(reps):
                _emit_body(
                    nc, tc, mybir, rep,
                    xt=xt, vone=vone, out_c=out_c,
                    wqk_sb=wqk_sb, wv_sb=wv_sb, wp_sb=wp_sb, tri_sb=tri_sb,
                    sel_sb=sel_sb, bqk_sb=bqk_sb, bout_sb=bout_sb,
                )

    nc.finalize()
    _prog_cache[key] = nc
    return nc


def _emit_body(nc, tc, mybir, rep, *, xt, vone, out_c, wqk_sb, wv_sb, wp_sb,
               tri_sb, sel_sb, bqk_sb, bout_sb):
    F32 = mybir.dt.float32
    F32R = mybir.dt.float32r
    AF = mybir.ActivationFunctionType
    MUL = mybir.AluOpType.mult
    R = f"r{rep}"

    with ExitStack() as persist:
        stP = persist.enter_context(tc.tile_pool(name=f"stP{R}", bufs=1))
        dpool = persist.enter_context(
            tc.tile_pool(name=f"dram{R}", bufs=1, space="DRAM"))
        # Q^T/K^T: m-tiles 0,1 = Q pairs; 2,3 = K pairs. [128, m, t]
        qk_sb = stP.tile([128, 4, T], F32R, name=f"qk_sb{R}")
        # V natural, 65-stride per head (65th col = ones)
        v_sb = stP.tile([128, NST, 260], F32R, name=f"v_sb{R}")
        # y raw + denominator row (partition 64), per t-chunk; blocks = head
        yraw_q = [
            stP.tile([65, 4, 512], F32R, name=f"yraw{R}_{q}")
            for q in range(NCH)
        ]
        y_in_q = [
            dpool.tile([256, 512], F32R, name=f"y_in{R}_{q}")
            for q in range(NCH)
        ]
        y_full_q = [
            dpool.tile([1024, 512], F32R, name=f"y_full{R}_{q}")
            for q in range(NCH)
        ]

        with (
            tc.tile_pool(name=f"stAB{R}", bufs=1) as stAB,
            tc.tile_pool(name=f"psA{R}", bufs=1, space="PSUM") as psA,
            tc.tile_pool(name=f"psS{R}", bufs=1, space="PSUM") as psS,
            tc.tile_pool(name=f"psY{R}", bufs=1, space="PSUM") as psY,
        ):
            xt_sb = stAB.tile([128, NKC, T], F32R, name=f"xt_sb{R}")
            vview = v_sb[:].rearrange("p t (h x) -> p t h x", h=4)
            nc.scalar.dma_start(vview[:, :, :, 64:65], vone[:])
            for n in range(NCH):
                nc.sync.dma_start(
                    xt_sb[:, :, n * 512:(n + 1) * 512],
                    xt[:, :, n * 512:(n + 1) * 512],
                )

            for n in range(NCH):
                    yraw = yraw_q[n]
                    for m in range(4):
                        ps = psA.tile([128, 512], F32, tag="pA", bufs=2,
                                      name=f"qkvps{R}_{n}_{m}")
                        for kc in range(NKC):
                            nc.tensor.matmul(
                                ps[:],
                                wqk_sb[:, kc, m * 128:(m + 1) * 128],
                                xt_sb[:, kc, n * 512:(n + 1) * 512],
                                start=(kc == 0), stop=(kc == NKC - 1),
                            )
                        if bqk_sb is not None:
                            nc.scalar.activation(
                                qk_sb[:, m, n * 512:(n + 1) * 512], ps[:],
                                AF.Copy, bias=bqk_sb[:, m:m + 1],
                            )
                        else:
                            nc.vector.tensor_copy(
                                qk_sb[:, m, n * 512:(n + 1) * 512], ps[:]
                            )
                    for tt in range(4 * n, 4 * n + 4):
                        psv = psA.tile([128, 512], F32, tag="pA", bufs=2,
                                       name=f"vps{R}_{tt}")
                        for kc in range(NKC):
                            nc.tensor.matmul(
                                psv[:, 0:256],
                                xt_sb[:, kc, tt * 128:(tt + 1) * 128],
                                wv_sb[:, kc, :],
                                start=(kc == 0), stop=(kc == NKC - 1),
                            )
                        nc.vector.tensor_copy(
                            v_sb[:, tt, :].rearrange(
                                "p (h x) -> p h x", h=4)[:, :, 0:64],
                            psv[:, 0:256].rearrange("p (h x) -> p h x", h=4),
                        )

                    # ---- stage B for chunk n ----
                    n_st = 4 * n + 4
                    for p in range(2):
                        ype = psY.tile([65, 512], F32, tag="ye", bufs=1,
                                       name=f"ype{R}_{n}_{p}")
                        ypo = psY.tile([65, 512], F32, tag="yo", bufs=1,
                                       name=f"ypo{R}_{n}_{p}")
                        for st in range(n_st):
                            diag = st - 4 * n
                            toff = 128 * diag if diag >= 0 else 0
                            scp = psS.tile([128, 1024], F32, tag="sc", bufs=2,
                                           name=f"scp{R}_{n}_{p}_{st}")
                            es = stAB.tile([128, 1024], F32R, tag="es", bufs=3,
                                           name=f"es{R}_{n}_{p}_{st}")
                            for hp in range(2):
                                pb = 64 * hp
                                nc.tensor.matmul(
                                    scp[:, hp * 512 + toff:(hp + 1) * 512],
                                    qk_sb[pb:pb + 64, 2 + p,
                                          st * 128:(st + 1) * 128],
                                    qk_sb[pb:pb + 64, p,
                                          n * 512 + toff:(n + 1) * 512],
                                    start=True, stop=True,
                                )
                            if diag < 0:
                                nc.scalar.activation(
                                    es[:], scp[:], AF.Exp, scale=0.125
                                )
                            else:
                                esv = es[:].rearrange(
                                    "p (hp u) -> p hp u", hp=2)
                                scv = scp[:].rearrange(
                                    "p (hp u) -> p hp u", hp=2)
                                nc.scalar.activation(
                                    esv[:, :, toff:512], scv[:, :, toff:512],
                                    AF.Exp, scale=0.125,
                                )
                                for hp in range(2):
                                    nc.vector.tensor_tensor(
                                        es[:, hp * 512 + toff:
                                           hp * 512 + toff + 128],
                                        es[:, hp * 512 + toff:
                                           hp * 512 + toff + 128],
                                        tri_sb[:], MUL,
                                    )
                            for hp, yp in ((0, ype), (1, ypo)):
                                h = 2 * p + hp
                                nc.tensor.matmul(
                                    yp[:, toff:512],
                                    v_sb[:, st, 65 * h:65 * h + 65],
                                    es[:, hp * 512 + toff:(hp + 1) * 512],
                                    start=(st == 0), stop=(st == n_st - 1),
                                )
                        for hp, yp in ((0, ype), (1, ypo)):
                            h = 2 * p + hp
                            nc.vector.tensor_copy(yraw[:, h, :], yp[:])

                    # ---- normalize + AllGather for this chunk ----
                    den4 = stAB.tile([4, 512], F32R, tag="den4", bufs=2,
                                     name=f"den4{R}_{n}")
                    nc.scalar.dma_start(den4[:], yraw[64:65, :, :])
                    rf = stAB.tile([4, 512], F32, tag="rf", bufs=2,
                                   name=f"rf{R}_{n}")
                    nc.vector.reciprocal_approx_fast(
                        rf[:], den4[:].bitcast(F32))
                    r4 = stAB.tile([4, 512], F32R, tag="r4", bufs=2,
                                   name=f"r4{R}_{n}")
                    nc.vector.tensor_copy(r4[:], rf[:])
                    for h in range(4):
                        rb = psA.tile([64, 512], F32, tag="pA", bufs=2,
                                      name=f"rb{R}_{n}_{h}")
                        nc.tensor.matmul(
                            rb[:], sel_sb[0:4, h, :], r4[:],
                            start=True, stop=True,
                        )
                        nc.vector.tensor_tensor(
                            yraw[0:64, h, :], yraw[0:64, h, :],
                            rb[:], MUL,
                        )
                    nc.scalar.dma_start(
                        y_in_q[n][:].rearrange("(h p) u -> p h u", p=64),
                        yraw[0:64, :, :],
                    )
                    nc.gpsimd.collective_compute(
                        "AllGather",
                        mybir.AluOpType.bypass,
                        replica_groups=[[0, 1, 2, 3], [4, 5, 6, 7]],
                        ins=[y_in_q[n][:]],
                        outs=[y_full_q[n][:]],
                    )

        # ------- proj per half -------
        with (
            tc.tile_pool(name=f"stC{R}", bufs=1) as stC,
            tc.tile_pool(name=f"psP{R}", bufs=1, space="PSUM") as psP,
        ):
            for q in range(NCH):
                pp0 = psP.tile([128, 512], F32, tag="pp0", bufs=2,
                               name=f"pp0{R}_{q}")
                pp1 = psP.tile([128, 512], F32, tag="pp1", bufs=2,
                               name=f"pp1{R}_{q}")
                for kc in range(NKC):
                    yf = stC.tile([128, 512], F32R, tag="yf", bufs=4,
                                  name=f"yf{R}_{q}_{kc}")
                    dma_eng = nc.sync if kc % 2 == 0 else nc.scalar
                    dma_eng.dma_start(
                        yf[:], y_full_q[q][kc * 128:(kc + 1) * 128, :]
                    )
                    for m2, pp in ((0, pp0), (1, pp1)):
                        nc.tensor.matmul(
                            pp[:],
                            wp_sb[:, kc, m2 * 128:(m2 + 1) * 128],
                            yf[:],
                            start=(kc == 0), stop=(kc == NKC - 1),
                        )
                out_sb = stC.tile([128, 2, 512], F32, tag="out_sb", bufs=2,
                                  name=f"out_sb{R}_{q}")
                for m2, pp in ((0, pp0), (1, pp1)):
                    if bout_sb is not None:
                        nc.scalar.activation(
                            out_sb[:, m2, :], pp[:], AF.Copy,
                            bias=bout_sb[:, m2:m2 + 1],
                        )
                    else:
                        nc.vector.tensor_copy(out_sb[:, m2, :], pp[:])
                nc.sync.dma_start(
                    out_c[:, q * 512:(q + 1) * 512].rearrange(
                        "(m p) t -> p m t", p=128),
                    out_sb[:],
                )


def _chunked(a):
    """(C, X) -> [128, C/128, X] contraction-chunked layout."""
    c, x = a.shape
    return np.ascontiguousarray(
        a.reshape(c // 128, 128, x).transpose(1, 0, 2)
    )


def make_in_maps(x, w_attn, b_attn, w_proj, b_proj):
    x = np.asarray(x, dtype=np.float32)
    w_attn = np.asarray(w_attn, dtype=np.float32)
    b_attn = np.asarray(b_attn, dtype=np.float32)
    w_proj = np.asarray(w_proj, dtype=np.float32)
    b_proj = np.asarray(b_proj, dtype=np.float32)

    qk_bias = bool(np.any(b_attn[: 2 * C] != 0))
    b_out_full = b_attn[2 * C:] @ w_proj + b_proj  # V bias folds through
    out_bias = bool(np.any(b_out_full != 0))

    tri_np = np.triu(np.ones((128, 128), np.float32))
    vone_np = np.ones((128, NST, 4, 1), np.float32)
    sel_np = np.zeros((16, 16, 64), np.float32)
    for i in range(16):
        sel_np[i, i, :] = 1.0
    xt_g = []
    for g in range(B):
        xt_g.append(_chunked(np.ascontiguousarray(x[g].T)))

    in_maps = []
    for core in range(NCORES):
        g, r = core // 4, core % 4
        h0 = r * HL
        qcols = slice(h0 * D, (h0 + HL) * D)
        kcols = slice(C + h0 * D, C + (h0 + HL) * D)
        vcols = slice(2 * C + h0 * D, 2 * C + (h0 + HL) * D)
        wqk_np = _chunked(np.concatenate(
            [w_attn[:, qcols], w_attn[:, kcols]], axis=1))
        wv_np = _chunked(np.ascontiguousarray(w_attn[:, vcols]))
        wp_np = _chunked(np.ascontiguousarray(
            w_proj[:, 256 * r: 256 * (r + 1)]))
        m = {
            "xt": xt_g[g],
            "wqk": wqk_np,
            "wv": wv_np,
            "wp": wp_np,
            "tri": tri_np,
            "vone": vone_np,
            "sel": sel_np,
        }
        if qk_bias:
            bq = np.concatenate([b_attn[qcols], b_attn[kcols]])  # (512,)
            m["bqk"] = np.ascontiguousarray(
                bq.reshape(4, 128).T.astype(np.float32))
        if out_bias:
            bo = b_out_full[256 * r: 256 * (r + 1)]
            m["bout"] = np.ascontiguousarray(
                bo.reshape(2, 128).T.astype(np.float32))
        in_maps.append(m)
    return in_maps, qk_bias, out_bias


def assemble_output(results):
    out = np.empty((B, T, C), dtype=np.float32)
    for core in range(NCORES):
        g, r = core // 4, core % 4
        out[g][:, 256 * r: 256 * (r + 1)] = results[core]["out_c"].T
    return out


def kernel(x, w_attn, b_attn, w_proj, b_proj):
    from concourse.bass_utils import run_bass_kernel_spmd

    in_maps, qk_bias, out_bias = make_in_maps(
        x, w_attn, b_attn, w_proj, b_proj)
    nc = build_program(reps=1, qk_bias=qk_bias, out_bias=out_bias)
    res = run_bass_kernel_spmd(nc, in_maps, list(range(NCORES)))
    return assemble_output(res.results)
